# revision 7
# baseline (speedup 1.0000x reference)
"""Trainium2 Bass kernel for nn_Bottleneck_DCNv3 (8-core SPMD).

Strategy: data-parallel over pixels (2 samples x 4 row-blocks of 16 rows, one
block per NeuronCore; per-core inputs are host-sliced shards, outputs are
host-concatenated -- no collectives).

The DCNv3 deformable bilinear gather + mask blend is reformulated as a banded
matrix multiply per 128-pixel out-tile: blend = M @ window(xpw), where
xpw = cv1_out @ (in_w @ out_w @ diag(bn2_scale)) is the output-projected image
and M's 81 nonzeros/row (9x9 bins) are per-pixel "tent" products
K[u,v] = sum_p softmax-mask_p * tent(offy_p - (u-dy_p)) * tent(offx_p - (v-dx_p)),
computed with replicated-column GEMMs + ACT ops, scattered into M in DRAM via
strided (diagonal) DMA descriptors, then consumed by TensorE after an xbar
DMA-transpose reload.

All matmul operands are bf16 (PSUM accumulation stays fp32); BN scales are
folded into conv/projection weights so BN+SiLU collapses to one ACT op.

Validity/borders are handled by a zero ring of width 4 around each shard and
a per-core interior mask that also carries the input_proj bias (extra GEMM
K-row), reproducing grid_sample zero-padding semantics exactly.
"""
import os
import sys
from contextlib import ExitStack

import numpy as np
import ml_dtypes

if '/opt/trn_rl_repo' not in sys.path:
    sys.path.insert(0, '/opt/trn_rl_repo')

import concourse.bass as bass
import concourse.bacc as bacc
import concourse.tile as tile
from concourse import mybir
from concourse._compat import with_exitstack
from concourse.bass_utils import run_bass_kernel_spmd
import concourse.bass_utils as _bu

_orig_run_command = _bu.run_command


def _patched_run_command(cmd, *a, **kw):
    # ldw-opt=true (used by the fp32r baseline) rejects some bf16 LDWEIGHTS
    # ("InstLdweights is not compatible with LDW optimization"); bf16 weights
    # get FWL from codegen regardless, so keep the default (false).
    return _orig_run_command(cmd, *a, **kw)


_bu.run_command = _patched_run_command

AF = mybir.ActivationFunctionType
OP = mybir.AluOpType
FP = mybir.dt.float32
BF = mybir.dt.bfloat16
BF_NP = ml_dtypes.bfloat16

C = 256
H = W = 64
N = 2
ROWS = 16
YR = 24
XR = 26
WPAD = 72
YF = YR * WPAD          # 1728
XF = XR * WPAD          # 1872
XBUF = 1 + XF + 7       # 1880
PX = ROWS * 64          # 1024
NT = 7
NB = 9
NTILE = PX // 128       # 8
QW = 768                # padded band row stride (window px 0..719)
PXC = 512
CHUNK = 432
LN_EPS = 1e-5

LAST_EXEC_NS = None
LAST_RESULTS = None


# ---------------------------------------------------------------- host prep
def host_consts(inputs):
    """Shared (core-independent) constant tensors."""
    f32 = lambda a: np.ascontiguousarray(a, np.float32)
    bf = lambda a: np.ascontiguousarray(a, np.float32).astype(BF_NP)
    cons = {}

    s1 = np.asarray(inputs['bn1_g']) / np.sqrt(np.asarray(inputs['bn1_v']) + 1e-5)
    w1 = np.asarray(inputs['w1'], np.float32) * s1[:, None, None, None]
    w1t = np.zeros((128, 9 * 2 * 256), np.float32)
    for tap in range(9):
        for cic in range(2):
            blk = w1[:, cic * 128:(cic + 1) * 128, tap // 3, tap % 3].T
            w1t[:, (tap * 2 + cic) * 256:(tap * 2 + cic + 1) * 256] = blk
    cons['w1ta'] = bf(w1t[:, :4 * 512])          # taps 0-3
    cons['w1tb'] = bf(w1t[:, 4 * 512:])          # taps 4-8
    b1 = inputs['bn1_b'] - inputs['bn1_m'] * s1
    cons['bn1b'] = f32(np.stack([b1[:128], b1[128:]], 1))

    s2 = np.asarray(inputs['bn2_g']) / np.sqrt(np.asarray(inputs['bn2_v']) + 1e-5)
    W2 = (np.asarray(inputs['in_w'], np.float32)
          @ np.asarray(inputs['out_w'], np.float32)) * s2[None, :]
    w2c = np.zeros((128, 2 * 256), np.float32)
    for cic in range(2):
        w2c[:, cic * 256:(cic + 1) * 256] = W2[cic * 128:(cic + 1) * 128, :]
    cons['w2c'] = bf(w2c)
    cons['inbw2'] = bf(((np.asarray(inputs['in_b']) @ np.asarray(inputs['out_w']))
                        * s2)[None, :])
    b2 = (np.asarray(inputs['bn2_b']) - np.asarray(inputs['bn2_m']) * s2
          + s2 * np.asarray(inputs['out_b']))
    cons['b2rep'] = bf(np.broadcast_to(b2[None, :], (128, 256)))

    dw = np.asarray(inputs['dw_w'], np.float32).reshape(C, 9)
    dwd = np.zeros((128, 2 * 9 * 128), np.float32)
    for g in range(2):
        for tap in range(9):
            np.fill_diagonal(dwd[:, (g * 9 + tap) * 128:(g * 9 + tap + 1) * 128],
                             dw[g * 128:(g + 1) * 128, tap])
    cons['dwd'] = bf(dwd)
    cons['dwb'] = f32(np.stack([inputs['dw_b'][:128], inputs['dw_b'][128:]], 1))
    cons['lng'] = f32(np.stack([inputs['ln_g'][:128], inputs['ln_g'][128:]], 1))
    cons['lnb'] = f32(np.stack([inputs['ln_b'][:128], inputs['ln_b'][128:]], 1))

    off_w = np.asarray(inputs['off_w'], np.float32)
    off_b = np.asarray(inputs['off_b'], np.float32)
    mk_w = np.asarray(inputs['mk_w'], np.float32)
    mk_b = np.asarray(inputs['mk_b'], np.float32)
    KS = np.arange(-3, 4, dtype=np.float32)

    wg1m63 = np.zeros((C, 63), np.float32)
    bg1m63 = np.zeros(63, np.float32)
    wg1t63 = np.zeros((C, 63), np.float32)
    bg1t63 = np.zeros(63, np.float32)
    for ky in range(NT):
        for pp in range(9):
            r = ky * 9 + pp
            wg1m63[:, r] = mk_w[:, pp]
            bg1m63[r] = mk_b[pp]
            wg1t63[:, r] = off_w[:, 2 * pp + 1]
            bg1t63[r] = off_b[2 * pp + 1] - KS[ky]
    wg1m = np.concatenate([wg1m63, wg1m63], 1)
    bg1m = np.concatenate([bg1m63, bg1m63])
    wg1t = np.concatenate([wg1t63, wg1t63], 1)
    bg1t = np.concatenate([bg1t63, bg1t63])
    wg1mp = np.zeros((128, 2 * 126), np.float32)
    wg1tp = np.zeros((128, 2 * 126), np.float32)
    for cic in range(2):
        wg1mp[:, cic * 126:(cic + 1) * 126] = wg1m[cic * 128:(cic + 1) * 128]
        wg1tp[:, cic * 126:(cic + 1) * 126] = wg1t[cic * 128:(cic + 1) * 128]
    cons['wg1m'] = bf(wg1mp)
    cons['wg1t'] = bf(wg1tp)
    cons['bg1m'] = f32(np.pad(bg1m, (0, 2))[:, None])
    cons['bg1t'] = f32(np.pad(bg1t, (0, 2))[:, None])

    wg2 = np.zeros((C, 441), np.float32)
    bg2 = np.zeros(441, np.float32)
    for kx in range(NT):
        for j in range(NT):
            for pp in range(9):
                r = kx * 63 + j * 9 + pp
                wg2[:, r] = off_w[:, 2 * pp]
                bg2[r] = off_b[2 * pp] - KS[kx]
    wg2p = np.zeros((128, 2 * 441), np.float32)
    for cic in range(2):
        wg2p[:, cic * 441:(cic + 1) * 441] = wg2[cic * 128:(cic + 1) * 128]
    cons['wg2'] = bf(wg2p)
    bg2p = np.zeros((128, 4), np.float32)
    for ch in range(4):
        c0, c1 = ch * 126, min(ch * 126 + 126, 441)
        bg2p[:c1 - c0, ch] = bg2[c0:c1]
    cons['bg2'] = bg2p

    S = np.zeros((441, 81), np.float32)
    for kx in range(NT):
        for ky in range(NT):
            for pp in range(9):
                dx, dy = pp // 3 - 1, pp % 3 - 1   # reference tap order
                u = dy + (ky - 3) + 4
                v = dx + (kx - 3) + 4
                S[kx * 63 + ky * 9 + pp, u * NB + v] = 1.0
    ssm = np.zeros((128, 4 * 81), np.float32)
    for ch in range(4):
        c0, c1 = ch * 126, min(ch * 126 + 126, 441)
        ssm[:c1 - c0, ch * 81:(ch + 1) * 81] = S[c0:c1]
    cons['ssm'] = bf(ssm)

    cons['onesA'] = bf(np.full((128, 1), 1.0 / C, np.float32))
    return cons


def core_inputs(x, n, r0):
    xs = np.zeros((C, XR, WPAD), np.float32)
    lo, hi = r0 - 5, r0 + 21
    clo, chi = max(lo, 0), min(hi, H)
    xs[:, clo - lo:chi - lo, 4:68] = x[n, :, clo:chi, :]
    xsh = np.zeros((C, XBUF), np.float32)
    xsh[:, 1:1 + XF] = xs.reshape(C, XF)
    ym = np.zeros((YR, WPAD), np.float32)
    for b in range(YR):
        if 0 <= r0 - 4 + b < H:
            ym[b, 4:68] = 1.0
    ymr = np.broadcast_to(ym.reshape(1, YF), (128, YF))
    xres = np.ascontiguousarray(
        np.transpose(x[n, :, r0:r0 + ROWS, :], (1, 2, 0)).reshape(PX, C),
        np.float32)
    return {'xsh': xsh.astype(BF_NP),
            'ymask': np.ascontiguousarray(ymr).astype(BF_NP),
            'xres': xres}


IN_SPECS = {
    'xsh': ((256, XBUF), BF), 'ymask': ((128, YF), BF), 'xres': ((PX, 256), FP),
    'w1ta': ((128, 2048), BF), 'w1tb': ((128, 2560), BF),
    'w2c': ((128, 512), BF), 'inbw2': ((1, 256), BF),
    'dwd': ((128, 2304), BF), 'dwb': ((128, 2), FP),
    'bn1b': ((128, 2), FP), 'lng': ((128, 2), FP), 'lnb': ((128, 2), FP),
    'b2rep': ((128, 256), BF),
    'wg1m': ((128, 252), BF), 'wg1t': ((128, 252), BF),
    'bg1m': ((128, 1), FP), 'bg1t': ((128, 1), FP),
    'wg2': ((128, 882), BF), 'bg2': ((128, 4), FP),
    'ssm': ((128, 324), BF), 'onesA': ((128, 1), BF),
}


# ---------------------------------------------------------------- kernel IR
@with_exitstack
def dcn_kernel(ctx: ExitStack, tc: tile.TileContext, outs, ins):
    nc = tc.nc
    NCH = YF // CHUNK      # 4
    out_dram = outs['out']

    cpool = ctx.enter_context(tc.tile_pool(name="consts", bufs=1))
    wpool = ctx.enter_context(tc.tile_pool(name="work", bufs=1))
    spool = ctx.enter_context(tc.tile_pool(name="small", bufs=2))
    ps_a = ctx.enter_context(tc.tile_pool(name="psa", bufs=4, space="PSUM"))
    ps_mm = ctx.enter_context(tc.tile_pool(name="psmm", bufs=3, space="PSUM"))
    ps_st = ctx.enter_context(tc.tile_pool(name="psst", bufs=1, space="PSUM"))

    def cload(name, dt=None, eng=None):
        shape, dtt = IN_SPECS[name]
        t = cpool.tile(list(shape), dtt, name=name, tag=name)
        (eng or nc.sync).dma_start(t[:], ins[name][:, :])
        return t

    x2 = []
    for g in range(2):
        t = wpool.tile([128, XBUF], BF, name=f'x2_{g}', tag=f'x2_{g}')
        nc.scalar.dma_start(t[:], ins['xsh'][g * 128:(g + 1) * 128, :])
        x2.append(t)
    w1ta = cload('w1ta')
    w1tb = cload('w1tb', eng=nc.scalar)
    bn1b = cload('bn1b')
    ymb = cload('ymask')
    w2c = cload('w2c', eng=nc.scalar)
    inbw2 = cload('inbw2')
    dwd = cload('dwd', eng=nc.scalar)
    dwb = cload('dwb')
    lng = cload('lng')
    lnb = cload('lnb')
    b2rep = cload('b2rep', eng=nc.scalar)
    wg1m = cload('wg1m')
    wg1t = cload('wg1t')
    bg1m = cload('bg1m')
    bg1t = cload('bg1t')
    wg2 = cload('wg2', eng=nc.scalar)
    bg2 = cload('bg2')
    ssm = cload('ssm')
    onesA = cload('onesA')

    ones_row = cpool.tile([1, 128], BF, name='ones_row', tag='ones_row')
    nc.gpsimd.memset(ones_row[:], 1.0)
    epsc = cpool.tile([128, 1], FP, name='epsc', tag='epsc')
    nc.gpsimd.memset(epsc[:], LN_EPS)
    onec = cpool.tile([128, 1], FP, name='onec', tag='onec')
    nc.gpsimd.memset(onec[:], 1.0)
    zeroc = cpool.tile([128, 1], FP, name='zeroc', tag='zeroc')
    nc.gpsimd.memset(zeroc[:], 0.0)

    xpw_pm = nc.dram_tensor('xpw_pm', [YF + 48, 256], BF, kind='Internal')
    mdram = nc.dram_tensor('mdram', [NTILE * 128 * QW], BF, kind='Internal')

    zero720 = cpool.tile([128, QW], BF, name='zero720', tag='zero720')
    nc.gpsimd.memset(zero720[:], 0.0)
    for t in range(NTILE):
        dstz = bass.AP(tensor=mdram, offset=t * 128 * QW, ap=[[QW, 128], [1, QW]])
        (nc.sync if t % 2 else nc.scalar).dma_start(out=dstz, in_=zero720[:])
    dstxz = bass.AP(tensor=xpw_pm, offset=YF * 256, ap=[[256, 48], [1, 256]])
    nc.sync.dma_start(out=dstxz, in_=zero720[0:48, 0:256])

    def w1slice(tap, cic, g):
        i = tap * 2 + cic
        if i < 8:
            return w1ta[:, i * 256 + g * 128: i * 256 + g * 128 + 128]
        i -= 8
        return w1tb[:, i * 256 + g * 128: i * 256 + g * 128 + 128]

    # ================= stage A: cv1 + BN/SiLU + ymask =================
    y_sb = [wpool.tile([128, YF], BF, name=f'y_{g}', tag=f'y_{g}') for g in range(2)]
    for g in range(2):
        for ch in range(NCH):
            co0 = ch * CHUNK
            acc = ps_a.tile([128, CHUNK], FP, name='acc', tag='psa')
            for tap in range(9):
                sh = (tap // 3) * WPAD + (tap % 3 - 1)
                for cic in range(2):
                    nc.tensor.matmul(
                        acc[:], lhsT=w1slice(tap, cic, g),
                        rhs=(x2[cic][:, 1 + sh + co0: 1 + sh + co0 + CHUNK]),
                        start=(tap == 0 and cic == 0),
                        stop=(tap == 8 and cic == 1))
            tmp = spool.tile([128, CHUNK], BF, name='atmp', tag='atmp', bufs=3)
            nc.scalar.activation(tmp[:], acc[:], AF.Silu, bias=bn1b[:, g:g + 1])
            nc.vector.tensor_tensor(y_sb[g][:, co0:co0 + CHUNK], tmp[:],
                                    ymb[:, co0:co0 + CHUNK], op=OP.mult)

    # ====== stage B: xpw (pixel-major) = y.T@W2' + mask-row (x) inbW2' ======
    stg = wpool.tile([128, 14 * 256], BF, name='stg', tag='stg')
    for b in range(14):
        p0 = b * 128
        w = min(128, YF - p0)
        zp = ps_mm.tile([128, 256], FP, name='zpB', tag='mm')
        for cic in range(2):
            nc.tensor.matmul(zp[0:w, :],
                             lhsT=(y_sb[cic][:, p0:p0 + w]),
                             rhs=(w2c[:, cic * 256:(cic + 1) * 256]),
                             start=(cic == 0), stop=False)
        nc.tensor.matmul(zp[0:w, :], lhsT=(ymb[0:1, p0:p0 + w]),
                         rhs=(inbw2[0:1, :]), start=False, stop=True)
        if b % 2:
            nc.scalar.copy(stg[0:w, b * 256:(b + 1) * 256], zp[0:w, :])
        else:
            nc.vector.tensor_copy(stg[0:w, b * 256:(b + 1) * 256], zp[0:w, :])
    s3 = stg[:].rearrange("p (b c) -> p b c", c=256)
    dstB = bass.AP(tensor=xpw_pm, offset=0,
                   ap=[[256, 128], [128 * 256, 13], [1, 256]])
    nc.sync.dma_start(out=dstB, in_=s3[:, 0:13, :])
    dstB2 = bass.AP(tensor=xpw_pm, offset=13 * 128 * 256,
                    ap=[[256, 64], [1, 256]])
    nc.sync.dma_start(out=dstB2, in_=stg[0:64, 13 * 256:14 * 256])

    # =========== stage C: dw conv + LN + GELU + tent kernels ===========
    def stage_c_chunk(pc):
        p0 = pc * PXC
        x1 = []
        sq = []
        for g in range(2):
            yr = y_sb[g][:].rearrange("p (r w) -> p r w", w=WPAD)
            x1p = ps_mm.tile([128, PXC], FP, name='x1p', tag='mm')
            for tap in range(9):
                ky, kx = tap // 3, tap % 3
                srcap = yr[:, 3 + ky + pc * 8:3 + ky + pc * 8 + 8,
                           3 + kx:3 + kx + 64]
                nc.tensor.matmul(
                    x1p[:], lhsT=dwd[:, (g * 9 + tap) * 128:(g * 9 + tap + 1) * 128],
                    rhs=srcap, start=(tap == 0), stop=(tap == 8))
            x1g = spool.tile([128, PXC], BF, name=f'x1_{g}', tag=f'x1_{g}', bufs=2)
            nc.scalar.activation(x1g[:], x1p[:], AF.Identity, bias=dwb[:, g:g + 1])
            sqg = spool.tile([128, PXC], BF, name=f'sq_{g}', tag=f'sq_{g}', bufs=2)
            nc.scalar.activation(sqg[:], x1p[:], AF.Square, bias=dwb[:, g:g + 1])
            x1.append(x1g)
            sq.append(sqg)

        stats = ps_st.tile([64, PXC], FP, name='stats', tag='st')
        mu = stats[0:1, :]
        sqm = stats[32:33, :]
        for g in range(2):
            nc.tensor.matmul(mu, lhsT=(onesA[:, :]), rhs=(x1[g][:, :]),
                             start=(g == 0), stop=(g == 1))
        for g in range(2):
            nc.tensor.matmul(sqm, lhsT=(onesA[:, :]), rhs=(sq[g][:, :]),
                             start=(g == 0), stop=(g == 1))
        mu_sb = spool.tile([1, PXC], FP, name='mu_sb', tag='mu_sb')
        nc.vector.tensor_copy(mu_sb[:], mu)
        mu2 = spool.tile([1, PXC], FP, name='mu2', tag='mu2')
        nc.vector.tensor_tensor(mu2[:], mu_sb[:], mu_sb[:], op=OP.mult)
        var = spool.tile([1, PXC], FP, name='var', tag='var')
        nc.vector.tensor_tensor(var[:], sqm, mu2[:], op=OP.subtract)
        sdb = spool.tile([1, PXC], FP, name='sdb', tag='sdb')
        nc.scalar.activation(sdb[:], var[:], AF.Sqrt, bias=epsc[0:1, :], scale=1.0)
        rsf = spool.tile([1, PXC], FP, name='rsf', tag='rsf')
        nc.vector.reciprocal_approx_fast(out=rsf[:], in_=sdb[:])
        rsb = spool.tile([1, PXC], BF, name='rsb', tag='rsb')
        nc.scalar.copy(rsb[:], rsf[:])
        nmr = spool.tile([1, PXC], BF, name='nmr', tag='nmr')
        nc.vector.scalar_tensor_tensor(out=nmr[:], in0=mu_sb[:], scalar=-1.0,
                                       in1=rsf[:], op0=OP.mult, op1=OP.mult)
        rb = ps_mm.tile([128, PXC], FP, name='rb', tag='mm')
        nc.tensor.matmul(rb[:], lhsT=(ones_row[0:1, :]), rhs=(rsb[:, :]),
                         start=True, stop=True)
        nb = ps_mm.tile([128, PXC], FP, name='nb', tag='mm')
        nc.tensor.matmul(nb[:], lhsT=(ones_row[0:1, :]), rhs=(nmr[:, :]),
                         start=True, stop=True)
        x1n = []
        for g in range(2):
            t1 = spool.tile([128, PXC], BF, name='t1', tag='t1', bufs=2)
            nc.vector.tensor_tensor(t1[:], x1[g][:, :], rb[:], op=OP.mult)
            t2 = spool.tile([128, PXC], BF, name='t2', tag='t2', bufs=2)
            nc.vector.tensor_tensor(t2[:], t1[:], nb[:], op=OP.add)
            xng = spool.tile([128, PXC], BF, name=f'xn_{g}', tag=f'xn_{g}', bufs=2)
            nc.scalar.activation(xng[:], t2[:], AF.Gelu,
                                 bias=lnb[:, g:g + 1], scale=lng[:, g:g + 1])
            x1n.append(xng)

        g1m = ps_mm.tile([126, PXC], FP, name='g1m', tag='mm')
        for cic in range(2):
            nc.tensor.matmul(g1m[:], lhsT=(wg1m[:, cic * 126:(cic + 1) * 126]),
                             rhs=(x1n[cic][:, :]),
                             start=(cic == 0), stop=(cic == 1))
        g1t = ps_mm.tile([126, PXC], FP, name='g1t', tag='mm')
        for cic in range(2):
            nc.tensor.matmul(g1t[:], lhsT=(wg1t[:, cic * 126:(cic + 1) * 126]),
                             rhs=(x1n[cic][:, :]),
                             start=(cic == 0), stop=(cic == 1))
        m_sb = spool.tile([126, PXC], BF, name='m_sb', tag='m_sb', bufs=2)
        nc.scalar.activation(m_sb[:], g1m[:], AF.Exp, bias=bg1m[0:126, :], scale=1.0)
        tyab = spool.tile([126, PXC], BF, name='tyab', tag='ttmp', bufs=3)
        nc.scalar.activation(tyab[:], g1t[:], AF.Abs, bias=bg1t[0:126, :], scale=1.0)
        ty = spool.tile([126, PXC], BF, name='ty', tag='ttmp', bufs=3)
        nc.scalar.activation(ty[:], tyab[:], AF.Relu, bias=onec[0:126, :], scale=-1.0)
        Aten = spool.tile([126, PXC], BF, name='Aten', tag='Aten', bufs=2)
        nc.vector.tensor_tensor(Aten[:], m_sb[:], ty[:], op=OP.mult)

        Ps = []
        for chn in range(4):
            r0c, r1c = chn * 126, min(chn * 126 + 126, 441)
            rows = r1c - r0c
            g2 = ps_mm.tile([126, PXC], FP, name='g2', tag='mm')
            for cic in range(2):
                nc.tensor.matmul(g2[0:rows, :],
                                 lhsT=(wg2[:, cic * 441 + r0c: cic * 441 + r1c]),
                                 rhs=(x1n[cic][:, :]),
                                 start=(cic == 0), stop=(cic == 1))
            txab = spool.tile([126, PXC], BF, name='txab', tag='ttmp', bufs=3)
            nc.scalar.activation(txab[0:rows, :], g2[0:rows, :], AF.Abs,
                                 bias=bg2[0:rows, chn:chn + 1], scale=1.0)
            tx = spool.tile([126, PXC], BF, name='tx', tag='ttmp', bufs=3)
            nc.scalar.activation(tx[0:rows, :], txab[0:rows, :], AF.Relu,
                                 bias=onec[0:rows, :], scale=-1.0)
            Pc = spool.tile([126, PXC], BF, name=f'P_{chn}', tag=f'P_{chn}', bufs=2)
            nc.vector.tensor_tensor(Pc[0:rows, :], Aten[0:rows, :], tx[0:rows, :],
                                    op=OP.mult)
            Ps.append((Pc, rows))

        for ti in range(4):
            t = pc * 4 + ti
            kps = ps_mm.tile([128, 81], FP, name='kps', tag='mm')
            for chn in range(4):
                Pc, rows = Ps[chn]
                nc.tensor.matmul(kps[:], lhsT=(Pc[0:rows, ti * 128:(ti + 1) * 128]),
                                 rhs=(ssm[0:rows, chn * 81:(chn + 1) * 81]),
                                 start=(chn == 0), stop=(chn == 3))
            kraw = spool.tile([128, 81], BF, name='kraw', tag='kraw', bufs=2)
            den = spool.tile([128, 1], FP, name='den', tag='den', bufs=2)
            nc.scalar.activation(kraw[:], kps[:], AF.Identity, bias=zeroc[:, :], accum_out=den[:])
            rec = spool.tile([128, 1], FP, name='rec', tag='rec', bufs=2)
            nc.vector.reciprocal_approx_fast(out=rec[:], in_=den[:])
            knt = spool.tile([128, 81], BF, name='knt', tag='knt', bufs=2)
            nc.vector.tensor_scalar(out=knt[:], in0=kraw[:], scalar1=rec[:, :],
                                    scalar2=None, op0=OP.mult)
            knr = knt[:].rearrange("p (u v) -> p u v", v=9)
            for rr in range(2):
                dst = bass.AP(tensor=mdram,
                              offset=t * 128 * QW + rr * (64 * QW + WPAD),
                              ap=[[QW + 1, 64], [WPAD, 9], [1, 9]])
                nc.scalar.dma_start(out=dst, in_=knr[rr * 64:rr * 64 + 64, :, :])

    # ================= stage D: blend + BN2/SiLU + residual =============
    def emit_blend(t):
        mt = spool.tile([128, 6 * 128], BF, name='mt', tag='mt', bufs=2)
        for qc in range(6):
            src = bass.AP(tensor=mdram, offset=t * 128 * QW + qc * 128,
                          ap=[[QW, 128], [1, 128]])
            nc.sync.dma_start_transpose(mt[:, qc * 128:(qc + 1) * 128], src)
        win = spool.tile([128, 6 * 256], BF, name='win', tag='win', bufs=2)
        winr = win[:].rearrange("p (b c) -> p b c", c=256)
        wsrc = bass.AP(tensor=xpw_pm, offset=t * 144 * 256,
                       ap=[[256, 128], [128 * 256, 6], [1, 256]])
        nc.scalar.dma_start(out=winr[:, 0:6, :], in_=wsrc)
        zpm = ps_mm.tile([128, 256], FP, name='zpm', tag='mm')
        for qc in range(6):
            nc.tensor.matmul(zpm[:],
                             lhsT=(mt[:, qc * 128:qc * 128 + 128]),
                             rhs=(winr[:, qc, :]),
                             start=(qc == 0), stop=(qc == 5))
        tb2 = spool.tile([128, 256], BF, name='tb2', tag='tb2', bufs=2)
        nc.vector.tensor_tensor(tb2[:], zpm[:], b2rep[:], op=OP.add)
        zs = spool.tile([128, 256], BF, name='zs', tag='zs', bufs=2)
        nc.scalar.activation(zs[:], tb2[:], AF.Silu, bias=zeroc[:, :])
        xrt = spool.tile([128, 256], FP, name='xrt', tag='xrt', bufs=2)
        nc.sync.dma_start(xrt[:], ins['xres'][t * 128:(t + 1) * 128, :])
        outt = spool.tile([128, 256], FP, name='outt', tag='outt', bufs=2)
        nc.vector.tensor_tensor(outt[:], zs[:], xrt[:], op=OP.add)
        nc.sync.dma_start(out=out_dram[t * 128:(t + 1) * 128, :], in_=outt[:])

    stage_c_chunk(0)
    for t in range(4):
        emit_blend(t)
    stage_c_chunk(1)
    for t in range(4, 8):
        emit_blend(t)


# ---------------------------------------------------------------- driver
_CACHED_NC = None


def _build_nc():
    global _CACHED_NC
    if _CACHED_NC is not None:
        return _CACHED_NC
    nc = bacc.Bacc("TRN2", target_bir_lowering=False, debug=False, num_devices=8)
    ins = {}
    for name, (shape, dtt) in IN_SPECS.items():
        ins[name] = nc.dram_tensor(name, list(shape), dtt, kind='ExternalInput').ap()
    out_ap = nc.dram_tensor('out', [PX, 256], FP, kind='ExternalOutput').ap()
    with nc.allow_low_precision(reason="bf16 matmul operands and intermediates"):
        with tile.TileContext(nc) as tc:
            dcn_kernel(tc, {'out': out_ap}, ins)
    nc.compile()
    _CACHED_NC = nc
    return nc


def kernel(**inputs):
    global LAST_EXEC_NS, LAST_RESULTS
    inputs = {k: np.asarray(v) for k, v in inputs.items()}
    x = np.asarray(inputs['x'], np.float32)
    cons = host_consts(inputs)
    in_maps = []
    shards = []
    for core in range(8):
        n, r0 = core // 4, (core % 4) * 16
        shards.append((n, r0))
        im = dict(cons)
        im.update(core_inputs(x, n, r0))
        in_maps.append(im)

    nc = _build_nc()
    res = run_bass_kernel_spmd(nc, in_maps, core_ids=list(range(8)))
    LAST_RESULTS = res
    LAST_EXEC_NS = res.exec_time_ns

    out = np.zeros((N, C, H, W), np.float32)
    for core, (n, r0) in enumerate(shards):
        blk = res.results[core]['out'].reshape(ROWS, 64, C)
        out[n, :, r0:r0 + 16, :] = np.transpose(blk, (2, 0, 1))
    return out
